# revision 14
# baseline (speedup 1.0000x reference)
"""Bass/Trainium2 kernel for the EvolutionAgentQuantum circuit.

10-qubit state-vector simulation, batch 4096, 5 layers of
[per-sample RY] -> [fused RZ diagonal] -> [shared RY] -> [CNOT ring],
then 4 Pauli-Z expectations. Data-parallel over 8 NeuronCores
(512 samples each), 4 partition-chains of 128 samples per core.

v2 (engine-balanced, fp16 state):
 - State held in fp16 [128, 2048] (re|im) per chain; all tensor_tensor
   ops run in the DVE 2x perf mode, PE transposes stream fp16.
 - Per-sample RY butterflies on DVE in tan-premultiply form
   (u = t*state via 4x tensor_scalar, then two 2x tensor_tensor ops);
   qubit 9 (stride-1 pairs, no 2x mode) as scalar_tensor_tensor.
 - All 20 RZ gates of a layer commute into one diagonal: phase table
   built by a single-sided doubling recursion on ScalarE (10 ops:
   seed -sum(phi), then +2*phi_j blocks), magic-round + Cody-Waite
   range reduction, Sin activations -> fp16 sin/cos tables, complex
   multiply split across engines: real part on DVE, imaginary part on
   Pool (concurrent independent op chains).
 - The phase pipeline for layer l+1 is emitted during layer l (it only
   depends on x), so ScalarE runs ahead of DVE/Pool.
 - Shared RY + CNOT ring = fixed 1024x1024 fp16 matmul per layer on
   the PE (state-stationary, 8-acc chains, per-layer double-buffered
   weight DMA). Per-layer cos-product rescale applied at PSUM evac.
 - Epilogue: probs = re^2+im^2 (fp16), transposed once, then one
   8-acc matmul against a [1024, 4] Z-sign table gives all four
   <Z_i> per sample directly.
"""

import sys
import os

for _p in ("/opt/trn_rl_repo", "/root/.axon_site/_ro/trn_rl_repo"):
    if os.path.isdir(_p) and _p not in sys.path:
        sys.path.insert(0, _p)

import numpy as np

import concourse.bacc as bacc
from concourse import mybir
from concourse.bass_utils import run_bass_kernel_spmd
from concourse.tile import TileContext

AF = mybir.ActivationFunctionType
ALU = mybir.AluOpType
F32 = mybir.dt.float32
F16 = mybir.dt.float16

N_CORES = 8
BATCH = 4096
OBS = 10
NQ = 10
NL = 5
NOUT = 4
PER_CORE = BATCH // N_CORES      # 512
P = 128
NCHAIN = PER_CORE // P           # 4
NAMP = 1 << NQ                   # 1024
SFREE = 2 * NAMP                 # 2048 (re | im)

PI = float(np.pi)
HALF_PI = float(np.pi / 2)
TWO_PI = float(2 * np.pi)
MAGIC = float(1.5 * 2 ** 23)
CW1 = 6.28125
CW2 = float(np.float32(2 * np.pi - CW1))
CW3 = float(2 * np.pi - CW1 - np.float32(2 * np.pi - CW1))

O_YS = 0
O_ZS = 50
O_ZW = 100
O_TW = 150
O_NTW = 200
O_CWP = 250
O_OS = 255
TBL_LEN = 259

N_DVE_Q = 6                      # qubits 0..5 halves on DVE, rest on Pool


def _bf_views(t, q):
    co = 1 << (q + 1)
    inner = 1 << (9 - q)
    return t[:].rearrange("p (co t i) -> p co t i", co=co, t=2, i=inner)


def _build_nc(rep=1):
    nc = bacc.Bacc()
    x_d = nc.declare_dram_parameter("x", [PER_CORE, OBS], F32, isOutput=False)
    tbl_d = nc.declare_dram_parameter("tbl", [TBL_LEN], F32, isOutput=False)
    wm_d = nc.declare_dram_parameter("wm", [NL, NAMP, NAMP], F16, isOutput=False)
    id_d = nc.declare_dram_parameter("ident", [P, P], F16, isOutput=False)
    zs_d = nc.declare_dram_parameter("zsign", [NAMP, NOUT], F16, isOutput=False)
    out_d = nc.declare_dram_parameter("out", [PER_CORE, NOUT], F32, isOutput=True)

    with TileContext(nc) as tc:
        with tc.tile_pool(name="pool", bufs=1) as pool, \
             tc.tile_pool(name="psum", bufs=2, space="PSUM") as psum, \
             tc.tile_pool(name="psum_mm", bufs=4, space="PSUM") as psum_mm:
            tbl = pool.tile([P, TBL_LEN], F32, tag="tbl")
            nc.sync.dma_start(
                out=tbl[:], in_=tbl_d[:].unsqueeze(0).to_broadcast((P, TBL_LEN))
            )
            ident = pool.tile([P, P], F16, tag="ident")
            nc.sync.dma_start(out=ident[:], in_=id_d[:])
            zsign = pool.tile([P, 8 * NOUT], F16, tag="zsign")
            nc.sync.dma_start(
                out=zsign[:].rearrange("p (r c) -> p r c", r=8),
                in_=zs_d[:].rearrange("(r p) c -> p r c", p=P),
            )
            wt = [[pool.tile([P, 4 * NAMP], F16, tag=f"wt{pp}{h}",
                             name=f"wt{pp}{h}")
                   for h in range(2)] for pp in range(2)]
            c_inv2pi = pool.tile([P, 1], F32, tag="c_inv2pi")
            c_magic = pool.tile([P, 1], F32, tag="c_magic")
            c_nmagic = pool.tile([P, 1], F32, tag="c_nmagic")
            nc.vector.memset(c_inv2pi[:], 1.0 / TWO_PI)
            nc.vector.memset(c_magic[:], MAGIC)
            nc.vector.memset(c_nmagic[:], -MAGIC)

            C = [dict() for _ in range(NCHAIN)]
            for ch in range(NCHAIN):
                tg = f"_{ch}"
                d = C[ch]
                xt = pool.tile([P, OBS], F32, tag="xt" + tg)
                nc.sync.dma_start(out=xt[:], in_=x_d[ch * P : (ch + 1) * P, :])

                ty = pool.tile([P, 5 * OBS], F32, tag="ty" + tg)
                sy = pool.tile([P, 5 * OBS], F32, tag="sy" + tg)
                cy = pool.tile([P, 5 * OBS], F32, tag="cy" + tg)
                rcy = pool.tile([P, 5 * OBS], F32, tag="rcy" + tg)
                nty = pool.tile([P, 5 * OBS], F32, tag="nty" + tg)
                ph = pool.tile([P, 5 * OBS], F32, tag="ph" + tg)
                ph2 = pool.tile([P, 5 * OBS], F32, tag="ph2" + tg)
                nsm = pool.tile([P, NL], F32, tag="nsm" + tg)
                clp = pool.tile([P, NL], F32, tag="clp" + tg)

                xb = xt[:].unsqueeze(1).to_broadcast((P, NL, OBS))
                tyv = ty[:].rearrange("p (l q) -> p l q", l=NL)
                ysv = tbl[:, O_YS : O_YS + 50].rearrange("p (l q) -> p l q", l=NL)
                nc.vector.tensor_tensor(tyv, xb, ysv, ALU.mult)
                nc.scalar.activation(sy[:], ty[:], AF.Sin)
                nc.vector.add_range_wrap(cy[:], ty[:], HALF_PI, PI, TWO_PI)
                nc.scalar.activation(cy[:], cy[:], AF.Sin)
                nc.vector.reciprocal(rcy[:], cy[:])
                nc.vector.tensor_tensor(ty[:], sy[:], rcy[:], ALU.mult)
                nc.vector.tensor_scalar_mul(nty[:], ty[:], -1.0)
                for l in range(NL):
                    nc.vector.tensor_reduce(
                        clp[:, l : l + 1], cy[:, 10 * l : 10 * l + 10],
                        mybir.AxisListType.X, ALU.mult,
                    )
                phv = ph[:].rearrange("p (l q) -> p l q", l=NL)
                zsv = tbl[:, O_ZS : O_ZS + 50].rearrange("p (l q) -> p l q", l=NL)
                nc.vector.tensor_tensor(phv, xb, zsv, ALU.mult)
                nc.vector.tensor_tensor(
                    ph[:], ph[:], tbl[:, O_ZW : O_ZW + 50], ALU.add
                )
                nc.vector.tensor_scalar_mul(ph2[:], ph[:], 2.0)
                for l in range(NL):
                    nc.vector.tensor_reduce(
                        nsm[:, l : l + 1], ph[:, 10 * l : 10 * l + 10],
                        mybir.AxisListType.X, ALU.add,
                    )
                nc.vector.tensor_scalar_mul(nsm[:], nsm[:], -1.0)

                d["ty"], d["nty"] = ty, nty
                d["ph2"], d["nsm"], d["clp"] = ph2, nsm, clp
                d["sa"] = pool.tile([P, SFREE], F16, tag="sa" + tg, name="sa" + tg)
                d["sb"] = pool.tile([P, SFREE], F16, tag="sb" + tg, name="sb" + tg)
                d["u"] = pool.tile([P, SFREE], F16, tag="u" + tg, name="u" + tg)
                d["stb"] = pool.tile([P, SFREE], F16, tag="stb" + tg,
                                     name="stb" + tg)
                d["pht"] = pool.tile([P, NAMP], F32, tag="pht" + tg,
                                     name="pht" + tg)
                d["mg"] = pool.tile([P, NAMP], F32, tag="mg" + tg,
                                    name="mg" + tg)
                d["rd"] = pool.tile([P, NAMP], F32, tag="rd" + tg,
                                    name="rd" + tg)
                # sin/cos double-buffered by layer parity (phase pipeline
                # for layer l+1 runs while cmul of layer l still reads l's)
                d["sinT"] = [pool.tile([P, NAMP], F16, tag=f"sinT{pp}" + tg,
                                       name=f"sinT{pp}" + tg) for pp in range(2)]
                d["cosT"] = [pool.tile([P, NAMP], F16, tag=f"cosT{pp}" + tg,
                                       name=f"cosT{pp}" + tg) for pp in range(2)]
                d["cur"], d["oth"] = d["sa"], d["sb"]

            def col(t, l, i):
                return t[:, 10 * l + i : 10 * l + i + 1]

            def emit_phase(l):
                # sin/cos tables for layer l's RZ diagonal; x-dependent only.
                pp = l % 2
                for ch in range(NCHAIN):
                    d = C[ch]
                    pht, mg, rd = d["pht"], d["mg"], d["rd"]
                    ph2, nsm = d["ph2"], d["nsm"]
                    sinT, cosT = d["sinT"][pp], d["cosT"][pp]
                    nc.scalar.activation(
                        pht[:, 0:1], nsm[:, l : l + 1], AF.Identity
                    )
                    for j in range(9, -1, -1):
                        g = 1 << (9 - j)
                        nc.scalar.activation(
                            pht[:, g : 2 * g], pht[:, 0:g], AF.Identity,
                            bias=col(ph2, l, j),
                        )
                    nc.scalar.activation(
                        mg[:], pht[:], AF.Identity,
                        scale=c_inv2pi[:], bias=c_magic[:],
                    )
                    nc.scalar.activation(mg[:], mg[:], AF.Identity,
                                         bias=c_nmagic[:])
                    nc.vector.cody_waite_cascade(
                        rd[:], pht[:], mg[:], CW1, CW2, CW3
                    )
                    nc.scalar.activation(sinT[:], rd[:], AF.Sin)
                    nc.vector.add_range_wrap(mg[:], rd[:], HALF_PI, PI, TWO_PI)
                    nc.scalar.activation(cosT[:], mg[:], AF.Sin)

            from contextlib import nullcontext
            loop_cm = tc.For_i(0, rep, 1) if rep > 1 else nullcontext()
            with loop_cm:
              for _rep in range(1):
                emit_phase(0)
                for l in range(NL):
                    pp = l % 2
                    nc.sync.dma_start(
                        out=wt[pp][0][:].rearrange("p (r m) -> p r m", r=4),
                        in_=wm_d[l, 0 : 4 * P].rearrange("(r p) m -> p r m", p=P),
                    )
                    nc.sync.dma_start(
                        out=wt[pp][1][:].rearrange("p (r m) -> p r m", r=4),
                        in_=wm_d[l, 4 * P : 8 * P].rearrange("(r p) m -> p r m", p=P),
                    )

                    # --- per-sample RY butterflies (all chains) ---
                    # u = tan*state on DVE (4x tensor_scalar); the two
                    # butterfly halves are 2x TT ops on DVE for qubits
                    # 0..N_DVE_Q-1 and Pool TT ops for the rest (Pool has
                    # no scalar_tensor_tensor on HW). Qubit-major emission
                    # so the engines pipeline across the four chains.
                    if l == 0:
                        for ch in range(NCHAIN):
                            d = C[ch]
                            cur = d["cur"]
                            ty = d["ty"]
                            nc.vector.memset(cur[:, 0:1], 1.0)
                            for j in range(9, -1, -1):
                                g = 1 << (9 - j)
                                nc.vector.tensor_scalar_mul(
                                    cur[:, g : 2 * g], cur[:, 0:g],
                                    col(ty, 0, j),
                                )
                    else:
                        for ch in range(NCHAIN):
                            d = C[ch]
                            cur, oth, u = d["cur"], d["oth"], d["u"]
                            ty = d["ty"]
                            nty = d["nty"]
                            for q in range(NQ):
                                cv = _bf_views(cur, q)
                                ov = _bf_views(oth, q)
                                if q == 9:
                                    nc.vector.scalar_tensor_tensor(
                                        ov[:, :, 0, :], cv[:, :, 1, :],
                                        col(nty, l, q), cv[:, :, 0, :],
                                        ALU.mult, ALU.add,
                                    )
                                    nc.vector.scalar_tensor_tensor(
                                        ov[:, :, 1, :], cv[:, :, 0, :],
                                        col(ty, l, q), cv[:, :, 1, :],
                                        ALU.mult, ALU.add,
                                    )
                                else:
                                    nc.vector.tensor_scalar_mul(
                                        u[:], cur[:], col(ty, l, q)
                                    )
                                    uv = _bf_views(u, q)
                                    nc.vector.tensor_tensor(
                                        ov[:, :, 0, :], cv[:, :, 0, :],
                                        uv[:, :, 1, :], ALU.subtract,
                                    )
                                    nc.vector.tensor_tensor(
                                        ov[:, :, 1, :], cv[:, :, 1, :],
                                        uv[:, :, 0, :], ALU.add,
                                    )
                                cur, oth = oth, cur
                            d["cur"], d["oth"] = cur, oth

                    # --- RZ diagonal: complex multiply by (cos, sin), on
                    # Pool so it overlaps the next chain's DVE butterflies ---
                    for ch in range(NCHAIN):
                        d = C[ch]
                        cur, oth, u = d["cur"], d["oth"], d["u"]
                        sinT, cosT = d["sinT"][pp], d["cosT"][pp]
                        sre = cur[:, 0:NAMP]
                        sim = cur[:, NAMP:SFREE]
                        dre = oth[:, 0:NAMP]
                        dim = oth[:, NAMP:SFREE]
                        uA = u[:, 0:NAMP]
                        uB = u[:, NAMP:SFREE]
                        if l == 0:
                            nc.vector.tensor_tensor(dre, sre, cosT[:], ALU.mult)
                            nc.gpsimd.tensor_tensor(dim, sre, sinT[:], ALU.mult)
                        else:
                            nc.vector.tensor_tensor(dre, sre, cosT[:], ALU.mult)
                            nc.vector.tensor_tensor(uA, sim, sinT[:], ALU.mult)
                            nc.vector.tensor_tensor(dre, dre, uA, ALU.subtract)
                            nc.gpsimd.tensor_tensor(dim, sre, sinT[:], ALU.mult)
                            nc.gpsimd.tensor_tensor(uB, sim, cosT[:], ALU.mult)
                            nc.gpsimd.tensor_tensor(dim, dim, uB, ALU.add)
                        d["cur"], d["oth"] = oth, cur

                    # --- phase pipeline for the next layer (runs ahead) ---
                    if l + 1 < NL:
                        emit_phase(l + 1)

                    # --- shared RY + CNOT ring matmul ---
                    for ch in range(NCHAIN):
                        d = C[ch]
                        cur, oth = d["cur"], d["oth"]
                        stb = d["stb"]
                        for comp in range(2):
                            for rg in range(2):
                                pt = psum.tile([P, 4 * P], F16, tag="ps_t",
                                               name="ps_t")
                                for rr in range(4):
                                    r = rg * 4 + rr
                                    nc.tensor.transpose(
                                        pt[:, rr * P : (rr + 1) * P],
                                        cur[:, comp * NAMP + r * P : comp * NAMP + (r + 1) * P],
                                        ident[:],
                                    )
                                nc.scalar.copy(
                                    stb[:, comp * NAMP + rg * 4 * P : comp * NAMP + (rg + 1) * 4 * P],
                                    pt[:],
                                )
                        for comp in range(2):
                            pm0 = psum_mm.tile([P, 4 * P], F32, tag="ps_mm",
                                               name="ps_mm")
                            pm1 = psum_mm.tile([P, 4 * P], F32, tag="ps_mm",
                                               name="ps_mm")
                            for r in range(8):
                                lhsT = stb[:, comp * NAMP + r * P : comp * NAMP + (r + 1) * P]
                                wh = wt[pp][0] if r < 4 else wt[pp][1]
                                rr = r % 4
                                nc.tensor.matmul(
                                    pm0[:], lhsT,
                                    wh[:, rr * NAMP : rr * NAMP + 4 * P],
                                    start=(r == 0), stop=(r == 7),
                                )
                                nc.tensor.matmul(
                                    pm1[:], lhsT,
                                    wh[:, rr * NAMP + 4 * P : (rr + 1) * NAMP],
                                    start=(r == 0), stop=(r == 7),
                                )
                            nc.scalar.activation(
                                cur[:, comp * NAMP : comp * NAMP + 4 * P],
                                pm0[:], AF.Identity,
                                scale=d["clp"][:, l : l + 1],
                            )
                            nc.scalar.activation(
                                cur[:, comp * NAMP + 4 * P : (comp + 1) * NAMP],
                                pm1[:], AF.Identity,
                                scale=d["clp"][:, l : l + 1],
                            )

            # --- epilogue: <Z_i> via one matmul against the sign table ---
            for ch in range(NCHAIN):
                tg = f"_{ch}"
                d = C[ch]
                cur, u, stb = d["cur"], d["u"], d["stb"]
                sre = cur[:, 0:NAMP]
                sim = cur[:, NAMP:SFREE]
                uA = u[:, 0:NAMP]
                uB = u[:, NAMP:SFREE]
                nc.vector.tensor_tensor(uA, sre, sre, ALU.mult)
                nc.vector.tensor_tensor(uB, sim, sim, ALU.mult)
                nc.vector.tensor_tensor(uA, uA, uB, ALU.add)
                for rg in range(2):
                    pt = psum.tile([P, 4 * P], F16, tag="ps_t", name="ps_t")
                    for rr in range(4):
                        r = rg * 4 + rr
                        nc.tensor.transpose(
                            pt[:, rr * P : (rr + 1) * P],
                            u[:, r * P : (r + 1) * P],
                            ident[:],
                        )
                    nc.scalar.copy(
                        stb[:, rg * 4 * P : (rg + 1) * 4 * P], pt[:]
                    )
                pz = psum.tile([P, NOUT], F32, tag="ps_z", name="ps_z")
                for r in range(8):
                    nc.tensor.matmul(
                        pz[:], stb[:, r * P : (r + 1) * P],
                        zsign[:, r * NOUT : (r + 1) * NOUT],
                        start=(r == 0), stop=(r == 7),
                    )
                ot = pool.tile([P, NOUT], F32, tag="ot" + tg)
                nc.scalar.copy(ot[:], pz[:])
                nc.vector.tensor_tensor(
                    ot[:], ot[:], tbl[:, O_OS : O_OS + NOUT], ALU.mult
                )
                nc.sync.dma_start(
                    out=out_d[ch * P : (ch + 1) * P, :], in_=ot[:]
                )

    nc.compile()
    return nc


def _host_table(input_scaling, weights, output_scaling):
    isc = np.asarray(input_scaling, np.float64)
    w = np.asarray(weights, np.float64)
    os_ = np.asarray(output_scaling, np.float64)
    ys = 0.5 * isc[:, :OBS]
    zs = 0.5 * isc[:, OBS : 2 * OBS]
    zw = 0.5 * w[:, :NQ]
    a = 0.5 * w[:, NQ : 2 * NQ]
    tw = np.tan(a)
    cwp = np.prod(np.cos(a), axis=1)
    tbl = np.concatenate(
        [ys.ravel(), zs.ravel(), zw.ravel(), tw.ravel(), (-tw).ravel(), cwp, os_]
    ).astype(np.float32)
    assert tbl.shape[0] == TBL_LEN
    return tbl


def _host_mats(weights):
    """Per-layer 1024x1024 lhsT matrices: W_l = (P_ring @ kron_i RY(w2_i)).T"""
    w = np.asarray(weights, np.float64)
    s = np.arange(NAMP)
    bits = [(s >> (9 - j)) & 1 for j in range(10)]
    for i in range(9):
        bits[i + 1] = bits[i + 1] ^ bits[i]
    bits[0] = bits[0] ^ bits[9]
    L = np.zeros(NAMP, np.int64)
    for j in range(10):
        L |= bits[j] << (9 - j)
    wm = np.empty((NL, NAMP, NAMP), np.float16)
    for l in range(NL):
        M = np.array([[1.0]])
        for i in range(NQ):
            a = 0.5 * w[l, NQ + i]
            c, sn = np.cos(a), np.sin(a)
            M = np.kron(M, np.array([[c, -sn], [sn, c]]))
        Ml = np.zeros_like(M)
        Ml[L, :] = M
        wm[l] = Ml.T.astype(np.float16)
    return wm


def _host_ident():
    return np.eye(P, dtype=np.float16)


def _host_zsign():
    a = np.arange(NAMP)
    cols = []
    for i in range(NOUT):
        b = (a >> (9 - i)) & 1
        cols.append((1 - 2 * b).astype(np.float16))
    return np.stack(cols, axis=1)


_CACHED = {}


def kernel(x, input_scaling, weights, output_scaling):
    x = np.ascontiguousarray(np.asarray(x, np.float32))
    tbl = _host_table(input_scaling, weights, output_scaling)
    wm = _host_mats(weights)
    ident = _host_ident()
    zsign = _host_zsign()

    if 1 not in _CACHED:
        _CACHED[1] = _build_nc(1)
    nc = _CACHED[1]

    in_maps = [
        {"x": x[c * PER_CORE : (c + 1) * PER_CORE], "tbl": tbl,
         "wm": wm, "ident": ident, "zsign": zsign}
        for c in range(N_CORES)
    ]
    res = run_bass_kernel_spmd(nc, in_maps, list(range(N_CORES))).results
    return np.concatenate([r["out"] for r in res], axis=0)


if __name__ == "__main__":
    rng = np.random.default_rng(0)
    x = rng.standard_normal((BATCH, OBS)).astype(np.float32)
    isc = np.ones((NL, 2 * NQ), np.float32)
    w = rng.uniform(-np.pi, np.pi, (NL, 2 * NQ)).astype(np.float32)
    os_ = np.ones((NOUT,), np.float32)
    out = kernel(x, isc, w, os_)
    print(out.shape, out[:2])
